# revision 14
# baseline (speedup 1.0000x reference)
"""Trainium2 Bass kernel for the 2-layer heterogeneous GCN encoder.

Strategy (8 NeuronCores, SPMD), v2:
  - Shard each relation's edges by dst-node owner: core k owns user rows
    [k*12500,(k+1)*12500) and item rows [k*6250,(k+1)*6250).
  - Aggregate-then-transform: segment_sum(x[src]*norm, dst) @ W computed as
    per-window PE matmuls  agg[fin, dstw] += gathered[e, fin].T @ S[e, dstw]
    with S[e, d] = (dstw_e == d) * norm_e built on DVE (fp16, 4x mode).
  - Gathers: one bulk SWDGE dma_gather per (dst-window x 32k src-range)
    (int16 local indices), fp16 tables -> ~0.3-2.4us of Pool time per
    thousands of rows instead of ~1us per 128 rows with indirect DMA.
  - Everything in the edge path is fp16 (tables, gathered rows, S, W);
    PSUM accumulation f32; final outputs f32.
  - Layer-1 outputs are written fp16, AllGathered (Shared DRAM), reused as
    layer-2 gather tables.

Self-contained: hardcodes problem shapes; host does only index-side prep.
"""

import os
import sys

sys.path.insert(0, "/opt/trn_rl_repo")

import numpy as np

import concourse.bass as bass
import concourse.bacc as bacc
import concourse.mybir as mybir
import concourse.tile as tile
from concourse.bass_utils import run_bass_kernel_spmd

P = 128
W = 256  # dst rows per aggregation window
RANGE = 32768  # max rows addressable by int16 gather indices
NCORES = 8
F16 = mybir.dt.float16
F32 = mybir.dt.float32
I16 = mybir.dt.int16

CFG = dict(N_U=100000, N_I=50000, E=1600000, D=128)

# relation -> (src table, dst type)
RELS = {
    "follows": ("user", "user"),
    "rates": ("user", "item"),
    "rev": ("item", "user"),
}


def _cdiv(a, b):
    return (a + b - 1) // b


class RelSched:
    """Harmonized per-(window, src-range) tile schedule for one relation."""

    def __init__(self, nwin, nrg):
        self.nwin = nwin
        self.nrg = nrg
        self.win = []  # per window: list of (rg, t0, T)
        self.Ttot = 0


def prep_relation(src, dst, n_src, n_dst, ncores=NCORES):
    """Shard edges by dst owner, sort by (dst window, src range), build the
    harmonized schedule and per-core packed arrays.

    Returns (sched, [(idx16, dstw, norm)] per core)."""
    shard = n_dst // ncores
    nwin = _cdiv(shard, W)
    nrg = _cdiv(n_src, RANGE)

    ones = np.ones_like(src, dtype=np.float64)
    deg_s = np.bincount(src, weights=ones, minlength=n_src)
    deg_d = np.bincount(dst, weights=ones, minlength=n_dst)
    inv_s = np.where(deg_s > 0, 1.0 / np.sqrt(deg_s), 0.0)
    inv_d = np.where(deg_d > 0, 1.0 / np.sqrt(deg_d), 0.0)
    norm = (inv_s[src] * inv_d[dst]).astype(np.float32)

    owner = dst // shard
    dloc = dst - owner * shard
    w_all = dloc // W
    dw_all = (dloc % W).astype(np.float32)
    rg_all = src // RANGE

    per_core = []
    counts = np.zeros((ncores, nwin * nrg), np.int64)
    for k in range(ncores):
        sel = owner == k
        s_k, dw_k, n_k = src[sel], dw_all[sel], norm[sel]
        cell = w_all[sel] * nrg + rg_all[sel]
        # sort by (cell, src): ascending gather addresses within each chunk
        order = np.lexsort((s_k, cell))
        s_k, dw_k, n_k, cell = s_k[order], dw_k[order], n_k[order], cell[order]
        counts[k] = np.bincount(cell, minlength=nwin * nrg)
        per_core.append((s_k, dw_k, n_k, cell))

    cmax = counts.max(axis=0).reshape(nwin, nrg)
    sched = RelSched(nwin, nrg)
    t0 = 0
    t0_cell = np.zeros((nwin, nrg), np.int64)
    for w in range(nwin):
        groups = []
        for rg in range(nrg):
            if cmax[w, rg] == 0:
                continue
            T = _cdiv(int(cmax[w, rg]), P)
            t0_cell[w, rg] = t0
            groups.append((rg, t0, T))
            t0 += T
        sched.win.append(groups)
    sched.Ttot = t0
    Ttot = t0

    packed = []
    for k in range(ncores):
        s_k, dw_k, n_k, cell = per_core[k]
        ck = counts[k]
        # position of each edge within its (w, rg) cell
        start = np.concatenate([[0], np.cumsum(ck)[:-1]])
        tok = np.arange(len(s_k)) - np.repeat(start, ck)
        # slot within the cell's tile block
        cw = cell // nrg
        crg = cell - cw * nrg
        base = t0_cell[cw, crg] * P
        slot = base + tok

        idx16 = np.zeros((16, Ttot * 8), np.int16)
        dstwA = np.full((P, Ttot), -1.0, np.float32)
        normA = np.zeros((P, Ttot), np.float32)
        loc = (s_k - crg * RANGE).astype(np.int16)
        idx16[slot % 16, slot // 16] = loc
        dstwA[slot % P, slot // P] = dw_k
        normA[slot % P, slot // P] = n_k
        packed.append((np.tile(idx16, (8, 1)), dstwA, normA))
    return sched, packed


def build_program(cfg, scheds):
    N_U, N_I, D = cfg["N_U"], cfg["N_I"], cfg["D"]
    SU, SI = N_U // NCORES, N_I // NCORES
    NWU, NWI = _cdiv(SU, W), _cdiv(SI, W)

    nc = bacc.Bacc("TRN2", target_bir_lowering=False)

    x_user = nc.dram_tensor("x_user", [N_U, D], F16, kind="ExternalInput")
    x_item = nc.dram_tensor("x_item", [N_I, D], F16, kind="ExternalInput")
    Ws = {
        n: nc.dram_tensor(n, [D, D], F16, kind="ExternalInput")
        for n in ["W1_follows", "W1_rates", "W1_rev", "W2_follows", "W2_rates", "W2_rev"]
    }
    bs = {
        n: nc.dram_tensor(n, [D], F32, kind="ExternalInput")
        for n in ["b1_follows", "b1_rates", "b1_rev", "b2_follows", "b2_rates", "b2_rev"]
    }
    iota_in = nc.dram_tensor("iotaw", [P, W], F16, kind="ExternalInput")
    ident16_in = nc.dram_tensor("ident16", [P, P], F16, kind="ExternalInput")
    ident32_in = nc.dram_tensor("ident32", [P, P], F32, kind="ExternalInput")
    streams = {}
    for r, (sched, _) in scheds.items():
        Ttot = sched.Ttot
        streams[r] = dict(
            idx=nc.dram_tensor(f"idx_{r}", [P, Ttot * 8], I16, kind="ExternalInput"),
            dstw=nc.dram_tensor(f"dstw_{r}", [P, Ttot], F32, kind="ExternalInput"),
            norm=nc.dram_tensor(f"norm_{r}", [P, Ttot], F32, kind="ExternalInput"),
        )
    out_user = nc.dram_tensor("out_user", [SU, D], F32, kind="ExternalOutput")
    out_item = nc.dram_tensor("out_item", [SI, D], F32, kind="ExternalOutput")

    with tile.TileContext(nc) as tc:
        with (
            tc.tile_pool(name="const", bufs=1) as cp,
            tc.tile_pool(name="gsl", bufs=3) as gp,
            tc.tile_pool(name="ixc", bufs=6) as ixp,
            tc.tile_pool(name="Sp", bufs=8) as sp,
            tc.tile_pool(name="agg", bufs=4) as aggp,
            tc.tile_pool(name="hb", bufs=3) as hp,
            tc.tile_pool(name="outp", bufs=4) as outp,
            tc.tile_pool(name="ps", bufs=3, space="PSUM") as pp,
            tc.tile_pool(name="psh", bufs=2, space="PSUM") as pph,
            tc.tile_pool(name="pstr", bufs=3, space="PSUM") as ptr,
            tc.tile_pool(name="dram", bufs=1, space="DRAM") as dp,
        ):
            # ---- constants ----
            iota_t = cp.tile([P, W], F16, tag="iota")
            nc.sync.dma_start(iota_t[:], iota_in[:])
            id16_t = cp.tile([P, P], F16, tag="id16")
            nc.sync.dma_start(id16_t[:], ident16_in[:])
            id32_t = cp.tile([P, P], F32, tag="id32")
            nc.sync.dma_start(id32_t[:], ident32_in[:])
            W_t = {}
            for n, Wd in Ws.items():
                W_t[n] = cp.tile([P, P], F16, tag=f"W_{n}", name=f"W_{n}")
                nc.sync.dma_start(W_t[n][:], Wd[:])
            b_t = {}
            for n, b in bs.items():
                b_t[n] = cp.tile([P, 1], F32, tag=f"b_{n}", name=f"bt_{n}")
                nc.sync.dma_start(b_t[n][:], b[:].unsqueeze(1))
            b1uv = cp.tile([P, 1], F32, tag="b1uv")
            nc.vector.tensor_tensor(
                out=b1uv[:], in0=b_t["b1_follows"][:], in1=b_t["b1_rev"][:],
                op=mybir.AluOpType.add,
            )
            nc.vector.tensor_scalar_mul(b1uv[:], b1uv[:], 0.5)
            b2uv = cp.tile([P, 1], F32, tag="b2uv")
            nc.vector.tensor_tensor(
                out=b2uv[:], in0=b_t["b2_follows"][:], in1=b_t["b2_rev"][:],
                op=mybir.AluOpType.add,
            )
            nc.vector.tensor_scalar_mul(b2uv[:], b2uv[:], 0.5)

            # ---- per-relation dstw/norm resident in SBUF ----
            st = {}
            for r, (sched, _) in scheds.items():
                Ttot = sched.Ttot
                st[r] = dict(
                    dstw=cp.tile([P, Ttot], F32, tag=f"dstw_{r}", name=f"dstwt_{r}"),
                    norm=cp.tile([P, Ttot], F32, tag=f"norm_{r}", name=f"normt_{r}"),
                )
                nc.sync.dma_start(st[r]["dstw"][:], streams[r]["dstw"][:])
                nc.sync.dma_start(st[r]["norm"][:], streams[r]["norm"][:])

            # ---- DRAM tiles for inter-layer tables (allocated per rep) ----

            ABL_NOS = os.environ.get("ABL_NOS") == "1"
            ABL_NOGATHER = os.environ.get("ABL_NOGATHER") == "1"
            ABL_NOMM = os.environ.get("ABL_NOMM") == "1"

            def agg_window(rel, w, table_ap, n_table):
                """Aggregate window w of relation rel into a PSUM tile
                agg[fin=128, W] = sum_e x[src_e] (x) onehot(dstw_e)*norm_e."""
                sched = scheds[rel][0]
                groups = sched.win[w]
                Twin = sum(T for (_, _, T) in groups)
                assert Twin > 0
                gsl = gp.tile([P, Twin, P], F16, tag="gsl")
                j0 = 0
                tlist = []
                for rg, t0g, T in groups:
                    ic = ixp.tile([P, T * 8], I16, tag="ixc")
                    nc.sync.dma_start(
                        ic[:], streams[rel]["idx"][:, t0g * 8 : (t0g + T) * 8]
                    )
                    r0 = rg * RANGE
                    r1 = min(r0 + RANGE, n_table)
                    sp_tiles = int(os.environ.get("ABL_SP", "0"))
                    if not ABL_NOGATHER:
                        if sp_tiles > 0:
                            # chunk into <=sp_tiles-tile pieces, single_packet
                            for c0 in range(0, T, sp_tiles):
                                c1 = min(c0 + sp_tiles, T)
                                nc.gpsimd.dma_gather(
                                    out_ap=gsl[:, j0 + c0 : j0 + c1, :],
                                    in_ap=table_ap[r0:r1, :],
                                    idxs_ap=ic[:, c0 * 8 : c1 * 8],
                                    num_idxs=(c1 - c0) * P,
                                    num_idxs_reg=(c1 - c0) * P,
                                    elem_size=P,
                                    single_packet=True,
                                )
                        else:
                            nc.gpsimd.dma_gather(
                                out_ap=gsl[:, j0 : j0 + T, :],
                                in_ap=table_ap[r0:r1, :],
                                idxs_ap=ic[:],
                                num_idxs=T * P,
                                num_idxs_reg=T * P,
                                elem_size=P,
                                single_packet=False,
                            )
                    for jj in range(T):
                        tlist.append((j0 + jj, t0g + jj))
                    j0 += T
                psum = pp.tile([P, W], F32, tag="aggps")
                for i, (j, t) in enumerate(tlist):
                    if ABL_NOS:
                        S = iota_t
                    else:
                        S = sp.tile([P, W], F16, tag="S")
                        nc.vector.tensor_scalar(
                            out=S[:],
                            in0=iota_t[:],
                            scalar1=st[rel]["dstw"][:, t : t + 1],
                            scalar2=st[rel]["norm"][:, t : t + 1],
                            op0=mybir.AluOpType.is_equal,
                            op1=mybir.AluOpType.mult,
                        )
                    if ABL_NOMM and not (i == 0 or i == len(tlist) - 1):
                        continue
                    nc.tensor.matmul(
                        out=psum[:],
                        lhsT=gsl[:, j, :],
                        rhs=S[:],
                        start=(i == 0),
                        stop=(i == len(tlist) - 1),
                    )
                return psum

            def write_windows(h_sb, w, nrows, dst_ap, ident_t, odt):
                """transpose h_sb [fout, nrows<=W] into [row, fout] blocks and
                DMA to dst_ap rows [w*W, w*W+nrows)."""
                for blk in range(_cdiv(nrows, P)):
                    r0, r1 = blk * P, min((blk + 1) * P, nrows)
                    ptile = ptr.tile([P, P], odt, tag="ptr")
                    nc.tensor.transpose(
                        out=ptile[: r1 - r0, :],
                        in_=h_sb[:, r0:r1],
                        identity=ident_t[:],
                    )
                    ob = outp.tile([P, P], odt, tag="ob")
                    nc.scalar.activation(
                        out=ob[: r1 - r0, :], in_=ptile[: r1 - r0, :],
                        func=mybir.ActivationFunctionType.Copy,
                    )
                    nc.sync.dma_start(
                        dst_ap[w * W + r0 : w * W + r1, :], ob[: r1 - r0, :]
                    )

            def user_layer(l, table_u, n_u, table_i, n_i, dst_ap, shard_rows, relu):
                Wf = W_t[f"W{l}_follows"]
                Wv = W_t[f"W{l}_rev"]
                bias = b1uv if l == 1 else b2uv
                nw = _cdiv(shard_rows, W)
                for w in range(nw):
                    nrows = min(W, shard_rows - w * W)
                    psF = agg_window("follows", w, table_u, n_u)
                    aggF = aggp.tile([P, W], F16, tag="aggF")
                    nc.scalar.activation(
                        out=aggF[:], in_=psF[:], func=mybir.ActivationFunctionType.Copy
                    )
                    psV = agg_window("rev", w, table_i, n_i)
                    aggV = aggp.tile([P, W], F16, tag="aggV")
                    nc.scalar.activation(
                        out=aggV[:], in_=psV[:], func=mybir.ActivationFunctionType.Copy
                    )
                    ph = pph.tile([P, W], F32, tag="hps")
                    nc.tensor.matmul(out=ph[:], lhsT=Wf[:], rhs=aggF[:], start=True, stop=False)
                    nc.tensor.matmul(out=ph[:], lhsT=Wv[:], rhs=aggV[:], start=False, stop=True)
                    if relu:
                        h_sb = hp.tile([P, W], F16, tag="h16")
                        nc.scalar.activation(
                            out=h_sb[:], in_=ph[:],
                            func=mybir.ActivationFunctionType.Relu,
                            bias=bias[:], scale=0.5,
                        )
                        write_windows(h_sb, w, nrows, dst_ap, id16_t, F16)
                    else:
                        h_sb = hp.tile([P, W], F32, tag="h32")
                        nc.vector.tensor_scalar(
                            out=h_sb[:], in0=ph[:],
                            scalar1=0.5, scalar2=bias[:],
                            op0=mybir.AluOpType.mult, op1=mybir.AluOpType.add,
                        )
                        write_windows(h_sb, w, nrows, dst_ap, id32_t, F32)

            def item_layer(l, table_u, n_u, dst_ap, shard_rows, relu):
                Wr = W_t[f"W{l}_rates"]
                bias = b_t[f"b{l}_rates"]
                nw = _cdiv(shard_rows, W)
                for w in range(nw):
                    nrows = min(W, shard_rows - w * W)
                    psR = agg_window("rates", w, table_u, n_u)
                    aggR = aggp.tile([P, W], F16, tag="aggR")
                    nc.scalar.activation(
                        out=aggR[:], in_=psR[:], func=mybir.ActivationFunctionType.Copy
                    )
                    ph = pph.tile([P, W], F32, tag="hps")
                    nc.tensor.matmul(out=ph[:], lhsT=Wr[:], rhs=aggR[:], start=True, stop=True)
                    if relu:
                        h_sb = hp.tile([P, W], F16, tag="h16")
                        nc.scalar.activation(
                            out=h_sb[:], in_=ph[:],
                            func=mybir.ActivationFunctionType.Relu,
                            bias=bias[:], scale=1.0,
                        )
                        write_windows(h_sb, w, nrows, dst_ap, id16_t, F16)
                    else:
                        h_sb = hp.tile([P, W], F32, tag="h32")
                        nc.vector.tensor_scalar(
                            out=h_sb[:], in0=ph[:],
                            scalar1=1.0, scalar2=bias[:],
                            op0=mybir.AluOpType.mult, op1=mybir.AluOpType.add,
                        )
                        write_windows(h_sb, w, nrows, dst_ap, id32_t, F32)

            ABL_NOAG = os.environ.get("ABL_NOAG") == "1"
            for _rep in range(int(os.environ.get("ABL_REPS", "1"))):
                u_slice = dp.tile([SU, D], F16, tag="u_slice", name=f"u_slice{_rep}")
                it_slice = dp.tile([SI, D], F16, tag="it_slice", name=f"it_slice{_rep}")
                u_full = dp.tile(
                    [N_U, D], F16, tag=f"u_full{_rep}", name=f"u_full{_rep}",
                    addr_space="Shared",
                )
                it_full = dp.tile(
                    [N_I, D], F16, tag=f"it_full{_rep}", name=f"it_full{_rep}",
                    addr_space="Shared",
                )
                # ---- layer 1 ----
                user_layer(1, x_user.ap(), N_U, x_item.ap(), N_I, u_slice[:], SU, relu=True)
                if not ABL_NOAG: nc.gpsimd.collective_compute(
                    "AllGather",
                    mybir.AluOpType.bypass,
                    replica_groups=[list(range(NCORES))],
                    ins=[u_slice[:]],
                    outs=[u_full[:]],
                )
                item_layer(1, x_user.ap(), N_U, it_slice[:], SI, relu=True)
                if not ABL_NOAG: nc.gpsimd.collective_compute(
                    "AllGather",
                    mybir.AluOpType.bypass,
                    replica_groups=[list(range(NCORES))],
                    ins=[it_slice[:]],
                    outs=[it_full[:]],
                )
                # ---- layer 2 (rates first: only needs u_full) ----
                item_layer(2, u_full[:], N_U, out_item.ap(), SI, relu=False)
                user_layer(2, u_full[:], N_U, it_full[:], N_I, out_user.ap(), SU, relu=False)

    nc.compile()
    return nc


def prepare(inputs):
    """Host-side prep + program build. Returns (nc, in_maps)."""
    cfg = dict(CFG)
    N_U = inputs["x_user"].shape[0]
    N_I = inputs["x_item"].shape[0]
    cfg.update(N_U=N_U, N_I=N_I, E=len(inputs["follows_src"]))

    rel_edges = {
        "follows": (inputs["follows_src"], inputs["follows_dst"], N_U, N_U),
        "rates": (inputs["rates_src"], inputs["rates_dst"], N_U, N_I),
        "rev": (inputs["rev_src"], inputs["rev_dst"], N_I, N_U),
    }
    scheds = {}
    for r, (s, d, ns, nd) in rel_edges.items():
        sched, packed = prep_relation(np.asarray(s), np.asarray(d), ns, nd)
        scheds[r] = (sched, packed)

    nc = build_program(cfg, scheds)

    iotaw = np.broadcast_to(np.arange(W, dtype=np.float16), (P, W)).copy()
    ident16 = np.eye(P, dtype=np.float16)
    ident32 = np.eye(P, dtype=np.float32)
    common = {}
    for n in ["x_user", "x_item",
              "W1_follows", "W1_rates", "W1_rev", "W2_follows", "W2_rates", "W2_rev"]:
        common[n] = np.asarray(inputs[n]).astype(np.float16)
    for n in ["b1_follows", "b1_rates", "b1_rev", "b2_follows", "b2_rates", "b2_rev"]:
        common[n] = np.asarray(inputs[n]).astype(np.float32)
    abl_idx0 = os.environ.get("ABL_IDX0") == "1"
    in_maps = []
    for k in range(NCORES):
        m = dict(common, iotaw=iotaw, ident16=ident16, ident32=ident32)
        for r in rel_edges:
            idx16, dstwA, normA = scheds[r][1][k]
            if abl_idx0:
                idx16 = np.zeros_like(idx16)
            m[f"idx_{r}"] = idx16
            m[f"dstw_{r}"] = dstwA
            m[f"norm_{r}"] = normA
        in_maps.append(m)
    return nc, in_maps


def assemble(results):
    u2 = np.concatenate([results[k]["out_user"] for k in range(NCORES)], axis=0)
    i2 = np.concatenate([results[k]["out_item"] for k in range(NCORES)], axis=0)
    return np.concatenate([u2, i2], axis=0)


def kernel(**inputs):
    nc, in_maps = prepare(inputs)
    res = run_bass_kernel_spmd(nc, in_maps, list(range(NCORES)))
    return assemble(res.results)


if __name__ == "__main__":
    pass


# revision 15
# speedup vs baseline: 1.1035x; 1.1035x over previous
"""Trainium2 Bass kernel for the 2-layer heterogeneous GCN encoder.

Strategy (8 NeuronCores, SPMD), v2:
  - Shard each relation's edges by dst-node owner: core k owns user rows
    [k*12500,(k+1)*12500) and item rows [k*6250,(k+1)*6250).
  - Aggregate-then-transform: segment_sum(x[src]*norm, dst) @ W computed as
    per-window PE matmuls  agg[fin, dstw] += gathered[e, fin].T @ S[e, dstw]
    with S[e, d] = (dstw_e == d) * norm_e built on DVE (fp16, 4x mode).
  - Gathers: one bulk SWDGE dma_gather per (dst-window x 32k src-range)
    (int16 local indices), fp16 tables -> ~0.3-2.4us of Pool time per
    thousands of rows instead of ~1us per 128 rows with indirect DMA.
  - Everything in the edge path is fp16 (tables, gathered rows, S, W);
    PSUM accumulation f32; final outputs f32.
  - Layer-1 outputs are written fp16, AllGathered (Shared DRAM), reused as
    layer-2 gather tables.

Self-contained: hardcodes problem shapes; host does only index-side prep.
"""

import os
import sys

sys.path.insert(0, "/opt/trn_rl_repo")

import numpy as np

import concourse.bass as bass
import concourse.bacc as bacc
import concourse.mybir as mybir
import concourse.tile as tile
from concourse.bass_utils import run_bass_kernel_spmd

P = 128
W = 256  # dst rows per aggregation window
RANGE = 32768  # max rows addressable by int16 gather indices
NCORES = 8
F16 = mybir.dt.float16
F32 = mybir.dt.float32
I16 = mybir.dt.int16

CFG = dict(N_U=100000, N_I=50000, E=1600000, D=128)

# relation -> (src table, dst type)
RELS = {
    "follows": ("user", "user"),
    "rates": ("user", "item"),
    "rev": ("item", "user"),
}


def _cdiv(a, b):
    return (a + b - 1) // b


class RelSched:
    """Harmonized per-(window, src-range) tile schedule for one relation."""

    def __init__(self, nwin, nrg):
        self.nwin = nwin
        self.nrg = nrg
        self.win = []  # per window: list of (rg, t0, T)
        self.Ttot = 0


def prep_relation(src, dst, n_src, n_dst, ncores=NCORES):
    """Shard edges by dst owner, sort by (dst window, src range), build the
    harmonized schedule and per-core packed arrays.

    Returns (sched, [(idx16, dstw, norm)] per core)."""
    shard = n_dst // ncores
    nwin = _cdiv(shard, W)
    nrg = _cdiv(n_src, RANGE)

    ones = np.ones_like(src, dtype=np.float64)
    deg_s = np.bincount(src, weights=ones, minlength=n_src)
    deg_d = np.bincount(dst, weights=ones, minlength=n_dst)
    inv_s = np.where(deg_s > 0, 1.0 / np.sqrt(deg_s), 0.0)
    inv_d = np.where(deg_d > 0, 1.0 / np.sqrt(deg_d), 0.0)
    norm = (inv_s[src] * inv_d[dst]).astype(np.float32)

    owner = dst // shard
    dloc = dst - owner * shard
    w_all = dloc // W
    dw_all = (dloc % W).astype(np.float32)
    rg_all = src // RANGE

    per_core = []
    counts = np.zeros((ncores, nwin * nrg), np.int64)
    for k in range(ncores):
        sel = owner == k
        s_k, dw_k, n_k = src[sel], dw_all[sel], norm[sel]
        cell = w_all[sel] * nrg + rg_all[sel]
        # sort by (cell, src): ascending gather addresses within each chunk
        order = np.lexsort((s_k, cell))
        s_k, dw_k, n_k, cell = s_k[order], dw_k[order], n_k[order], cell[order]
        counts[k] = np.bincount(cell, minlength=nwin * nrg)
        per_core.append((s_k, dw_k, n_k, cell))

    cmax = counts.max(axis=0).reshape(nwin, nrg)
    sched = RelSched(nwin, nrg)
    t0 = 0
    t0_cell = np.zeros((nwin, nrg), np.int64)
    for w in range(nwin):
        groups = []
        for rg in range(nrg):
            if cmax[w, rg] == 0:
                continue
            T = _cdiv(int(cmax[w, rg]), P)
            t0_cell[w, rg] = t0
            groups.append((rg, t0, T))
            t0 += T
        sched.win.append(groups)
    sched.Ttot = t0
    Ttot = t0

    packed = []
    for k in range(ncores):
        s_k, dw_k, n_k, cell = per_core[k]
        ck = counts[k]
        # position of each edge within its (w, rg) cell
        start = np.concatenate([[0], np.cumsum(ck)[:-1]])
        tok = np.arange(len(s_k)) - np.repeat(start, ck)
        # slot within the cell's tile block
        cw = cell // nrg
        crg = cell - cw * nrg
        base = t0_cell[cw, crg] * P
        slot = base + tok

        idx16 = np.zeros((16, Ttot * 8), np.int16)
        dstwA = np.full((P, Ttot), -1.0, np.float32)
        normA = np.zeros((P, Ttot), np.float32)
        loc = (s_k - crg * RANGE).astype(np.int16)
        idx16[slot % 16, slot // 16] = loc
        dstwA[slot % P, slot // P] = dw_k
        normA[slot % P, slot // P] = n_k
        packed.append((np.tile(idx16, (8, 1)), dstwA, normA))
    return sched, packed


def build_program(cfg, scheds):
    N_U, N_I, D = cfg["N_U"], cfg["N_I"], cfg["D"]
    SU, SI = N_U // NCORES, N_I // NCORES
    NWU, NWI = _cdiv(SU, W), _cdiv(SI, W)

    nq = int(os.environ.get("ABL_NQ", "1"))
    nc = bacc.Bacc("TRN2", target_bir_lowering=False, num_swdge_queues=nq)

    x_user = nc.dram_tensor("x_user", [N_U, D], F16, kind="ExternalInput")
    x_item = nc.dram_tensor("x_item", [N_I, D], F16, kind="ExternalInput")
    Ws = {
        n: nc.dram_tensor(n, [D, D], F16, kind="ExternalInput")
        for n in ["W1_follows", "W1_rates", "W1_rev", "W2_follows", "W2_rates", "W2_rev"]
    }
    bs = {
        n: nc.dram_tensor(n, [D], F32, kind="ExternalInput")
        for n in ["b1_follows", "b1_rates", "b1_rev", "b2_follows", "b2_rates", "b2_rev"]
    }
    iota_in = nc.dram_tensor("iotaw", [P, W], F16, kind="ExternalInput")
    ident16_in = nc.dram_tensor("ident16", [P, P], F16, kind="ExternalInput")
    ident32_in = nc.dram_tensor("ident32", [P, P], F32, kind="ExternalInput")
    streams = {}
    for r, (sched, _) in scheds.items():
        Ttot = sched.Ttot
        streams[r] = dict(
            idx=nc.dram_tensor(f"idx_{r}", [P, Ttot * 8], I16, kind="ExternalInput"),
            dstw=nc.dram_tensor(f"dstw_{r}", [P, Ttot], F32, kind="ExternalInput"),
            norm=nc.dram_tensor(f"norm_{r}", [P, Ttot], F32, kind="ExternalInput"),
        )
    out_user = nc.dram_tensor("out_user", [SU, D], F32, kind="ExternalOutput")
    out_item = nc.dram_tensor("out_item", [SI, D], F32, kind="ExternalOutput")

    with tile.TileContext(nc) as tc:
        with (
            tc.tile_pool(name="const", bufs=1) as cp,
            tc.tile_pool(name="gsl", bufs=3) as gp,
            tc.tile_pool(name="ixc", bufs=6) as ixp,
            tc.tile_pool(name="Sp", bufs=8) as sp,
            tc.tile_pool(name="agg", bufs=4) as aggp,
            tc.tile_pool(name="hb", bufs=3) as hp,
            tc.tile_pool(name="outp", bufs=4) as outp,
            tc.tile_pool(name="ps", bufs=3, space="PSUM") as pp,
            tc.tile_pool(name="psh", bufs=2, space="PSUM") as pph,
            tc.tile_pool(name="pstr", bufs=3, space="PSUM") as ptr,
            tc.tile_pool(name="dram", bufs=1, space="DRAM") as dp,
        ):
            # ---- constants ----
            iota_t = cp.tile([P, W], F16, tag="iota")
            nc.sync.dma_start(iota_t[:], iota_in[:])
            id16_t = cp.tile([P, P], F16, tag="id16")
            nc.sync.dma_start(id16_t[:], ident16_in[:])
            id32_t = cp.tile([P, P], F32, tag="id32")
            nc.sync.dma_start(id32_t[:], ident32_in[:])
            W_t = {}
            for n, Wd in Ws.items():
                W_t[n] = cp.tile([P, P], F16, tag=f"W_{n}", name=f"W_{n}")
                nc.sync.dma_start(W_t[n][:], Wd[:])
            b_t = {}
            for n, b in bs.items():
                b_t[n] = cp.tile([P, 1], F32, tag=f"b_{n}", name=f"bt_{n}")
                nc.sync.dma_start(b_t[n][:], b[:].unsqueeze(1))
            b1uv = cp.tile([P, 1], F32, tag="b1uv")
            nc.vector.tensor_tensor(
                out=b1uv[:], in0=b_t["b1_follows"][:], in1=b_t["b1_rev"][:],
                op=mybir.AluOpType.add,
            )
            nc.vector.tensor_scalar_mul(b1uv[:], b1uv[:], 0.5)
            b2uv = cp.tile([P, 1], F32, tag="b2uv")
            nc.vector.tensor_tensor(
                out=b2uv[:], in0=b_t["b2_follows"][:], in1=b_t["b2_rev"][:],
                op=mybir.AluOpType.add,
            )
            nc.vector.tensor_scalar_mul(b2uv[:], b2uv[:], 0.5)

            # ---- per-relation dstw/norm resident in SBUF ----
            st = {}
            for r, (sched, _) in scheds.items():
                Ttot = sched.Ttot
                st[r] = dict(
                    dstw=cp.tile([P, Ttot], F32, tag=f"dstw_{r}", name=f"dstwt_{r}"),
                    norm=cp.tile([P, Ttot], F32, tag=f"norm_{r}", name=f"normt_{r}"),
                )
                nc.sync.dma_start(st[r]["dstw"][:], streams[r]["dstw"][:])
                nc.sync.dma_start(st[r]["norm"][:], streams[r]["norm"][:])

            # ---- DRAM tiles for inter-layer tables (allocated per rep) ----

            ABL_NOS = os.environ.get("ABL_NOS") == "1"
            ABL_NOGATHER = os.environ.get("ABL_NOGATHER") == "1"
            ABL_NOMM = os.environ.get("ABL_NOMM") == "1"
            qctr = [0]

            def agg_window(rel, w, table_ap, n_table):
                """Aggregate window w of relation rel into a PSUM tile
                agg[fin=128, W] = sum_e x[src_e] (x) onehot(dstw_e)*norm_e."""
                sched = scheds[rel][0]
                groups = sched.win[w]
                Twin = sum(T for (_, _, T) in groups)
                assert Twin > 0
                gsl = gp.tile([P, Twin, P], F16, tag="gsl")
                j0 = 0
                tlist = []
                for rg, t0g, T in groups:
                    ic = ixp.tile([P, T * 8], I16, tag="ixc")
                    nc.sync.dma_start(
                        ic[:], streams[rel]["idx"][:, t0g * 8 : (t0g + T) * 8]
                    )
                    r0 = rg * RANGE
                    r1 = min(r0 + RANGE, n_table)
                    sp_tiles = int(os.environ.get("ABL_SP", "0"))
                    if not ABL_NOGATHER:
                        if sp_tiles > 0:
                            # chunk into <=sp_tiles-tile pieces, single_packet
                            for c0 in range(0, T, sp_tiles):
                                c1 = min(c0 + sp_tiles, T)
                                nc.gpsimd.dma_gather(
                                    out_ap=gsl[:, j0 + c0 : j0 + c1, :],
                                    in_ap=table_ap[r0:r1, :],
                                    idxs_ap=ic[:, c0 * 8 : c1 * 8],
                                    num_idxs=(c1 - c0) * P,
                                    num_idxs_reg=(c1 - c0) * P,
                                    elem_size=P,
                                    single_packet=True,
                                )
                        else:
                            qctr[0] = (qctr[0] + 1) % nq
                            nc.gpsimd.dma_gather(
                                out_ap=gsl[:, j0 : j0 + T, :],
                                in_ap=table_ap[r0:r1, :],
                                idxs_ap=ic[:],
                                num_idxs=T * P,
                                num_idxs_reg=T * P,
                                elem_size=P,
                                single_packet=False,
                                queue_num=qctr[0],
                            )
                    for jj in range(T):
                        tlist.append((j0 + jj, t0g + jj))
                    j0 += T
                psum = pp.tile([P, W], F32, tag="aggps")
                for i, (j, t) in enumerate(tlist):
                    if ABL_NOS:
                        S = iota_t
                    else:
                        S = sp.tile([P, W], F16, tag="S")
                        nc.vector.tensor_scalar(
                            out=S[:],
                            in0=iota_t[:],
                            scalar1=st[rel]["dstw"][:, t : t + 1],
                            scalar2=st[rel]["norm"][:, t : t + 1],
                            op0=mybir.AluOpType.is_equal,
                            op1=mybir.AluOpType.mult,
                        )
                    if ABL_NOMM and not (i == 0 or i == len(tlist) - 1):
                        continue
                    nc.tensor.matmul(
                        out=psum[:],
                        lhsT=gsl[:, j, :],
                        rhs=S[:],
                        start=(i == 0),
                        stop=(i == len(tlist) - 1),
                    )
                return psum

            def write_windows(h_sb, w, nrows, dst_ap, ident_t, odt):
                """transpose h_sb [fout, nrows<=W] into [row, fout] blocks and
                DMA to dst_ap rows [w*W, w*W+nrows)."""
                for blk in range(_cdiv(nrows, P)):
                    r0, r1 = blk * P, min((blk + 1) * P, nrows)
                    ptile = ptr.tile([P, P], odt, tag="ptr")
                    nc.tensor.transpose(
                        out=ptile[: r1 - r0, :],
                        in_=h_sb[:, r0:r1],
                        identity=ident_t[:],
                    )
                    ob = outp.tile([P, P], odt, tag="ob")
                    nc.scalar.activation(
                        out=ob[: r1 - r0, :], in_=ptile[: r1 - r0, :],
                        func=mybir.ActivationFunctionType.Copy,
                    )
                    nc.sync.dma_start(
                        dst_ap[w * W + r0 : w * W + r1, :], ob[: r1 - r0, :]
                    )

            def user_layer(l, table_u, n_u, table_i, n_i, dst_ap, shard_rows, relu):
                Wf = W_t[f"W{l}_follows"]
                Wv = W_t[f"W{l}_rev"]
                bias = b1uv if l == 1 else b2uv
                nw = _cdiv(shard_rows, W)
                for w in range(nw):
                    nrows = min(W, shard_rows - w * W)
                    psF = agg_window("follows", w, table_u, n_u)
                    aggF = aggp.tile([P, W], F16, tag="aggF")
                    nc.scalar.activation(
                        out=aggF[:], in_=psF[:], func=mybir.ActivationFunctionType.Copy
                    )
                    psV = agg_window("rev", w, table_i, n_i)
                    aggV = aggp.tile([P, W], F16, tag="aggV")
                    nc.scalar.activation(
                        out=aggV[:], in_=psV[:], func=mybir.ActivationFunctionType.Copy
                    )
                    ph = pph.tile([P, W], F32, tag="hps")
                    nc.tensor.matmul(out=ph[:], lhsT=Wf[:], rhs=aggF[:], start=True, stop=False)
                    nc.tensor.matmul(out=ph[:], lhsT=Wv[:], rhs=aggV[:], start=False, stop=True)
                    if relu:
                        h_sb = hp.tile([P, W], F16, tag="h16")
                        nc.scalar.activation(
                            out=h_sb[:], in_=ph[:],
                            func=mybir.ActivationFunctionType.Relu,
                            bias=bias[:], scale=0.5,
                        )
                        write_windows(h_sb, w, nrows, dst_ap, id16_t, F16)
                    else:
                        h_sb = hp.tile([P, W], F32, tag="h32")
                        nc.vector.tensor_scalar(
                            out=h_sb[:], in0=ph[:],
                            scalar1=0.5, scalar2=bias[:],
                            op0=mybir.AluOpType.mult, op1=mybir.AluOpType.add,
                        )
                        write_windows(h_sb, w, nrows, dst_ap, id32_t, F32)

            def item_layer(l, table_u, n_u, dst_ap, shard_rows, relu):
                Wr = W_t[f"W{l}_rates"]
                bias = b_t[f"b{l}_rates"]
                nw = _cdiv(shard_rows, W)
                for w in range(nw):
                    nrows = min(W, shard_rows - w * W)
                    psR = agg_window("rates", w, table_u, n_u)
                    aggR = aggp.tile([P, W], F16, tag="aggR")
                    nc.scalar.activation(
                        out=aggR[:], in_=psR[:], func=mybir.ActivationFunctionType.Copy
                    )
                    ph = pph.tile([P, W], F32, tag="hps")
                    nc.tensor.matmul(out=ph[:], lhsT=Wr[:], rhs=aggR[:], start=True, stop=True)
                    if relu:
                        h_sb = hp.tile([P, W], F16, tag="h16")
                        nc.scalar.activation(
                            out=h_sb[:], in_=ph[:],
                            func=mybir.ActivationFunctionType.Relu,
                            bias=bias[:], scale=1.0,
                        )
                        write_windows(h_sb, w, nrows, dst_ap, id16_t, F16)
                    else:
                        h_sb = hp.tile([P, W], F32, tag="h32")
                        nc.vector.tensor_scalar(
                            out=h_sb[:], in0=ph[:],
                            scalar1=1.0, scalar2=bias[:],
                            op0=mybir.AluOpType.mult, op1=mybir.AluOpType.add,
                        )
                        write_windows(h_sb, w, nrows, dst_ap, id32_t, F32)

            ABL_NOAG = os.environ.get("ABL_NOAG") == "1"
            for _rep in range(int(os.environ.get("ABL_REPS", "1"))):
                u_slice = dp.tile([SU, D], F16, tag="u_slice", name=f"u_slice{_rep}")
                it_slice = dp.tile([SI, D], F16, tag="it_slice", name=f"it_slice{_rep}")
                u_full = dp.tile(
                    [N_U, D], F16, tag=f"u_full{_rep}", name=f"u_full{_rep}",
                    addr_space="Shared",
                )
                it_full = dp.tile(
                    [N_I, D], F16, tag=f"it_full{_rep}", name=f"it_full{_rep}",
                    addr_space="Shared",
                )
                # ---- layer 1 ----
                user_layer(1, x_user.ap(), N_U, x_item.ap(), N_I, u_slice[:], SU, relu=True)
                if not ABL_NOAG: nc.gpsimd.collective_compute(
                    "AllGather",
                    mybir.AluOpType.bypass,
                    replica_groups=[list(range(NCORES))],
                    ins=[u_slice[:]],
                    outs=[u_full[:]],
                )
                item_layer(1, x_user.ap(), N_U, it_slice[:], SI, relu=True)
                if not ABL_NOAG: nc.gpsimd.collective_compute(
                    "AllGather",
                    mybir.AluOpType.bypass,
                    replica_groups=[list(range(NCORES))],
                    ins=[it_slice[:]],
                    outs=[it_full[:]],
                )
                # ---- layer 2 (rates first: only needs u_full) ----
                item_layer(2, u_full[:], N_U, out_item.ap(), SI, relu=False)
                user_layer(2, u_full[:], N_U, it_full[:], N_I, out_user.ap(), SU, relu=False)

    nc.compile()
    return nc


def prepare(inputs):
    """Host-side prep + program build. Returns (nc, in_maps)."""
    cfg = dict(CFG)
    N_U = inputs["x_user"].shape[0]
    N_I = inputs["x_item"].shape[0]
    cfg.update(N_U=N_U, N_I=N_I, E=len(inputs["follows_src"]))

    rel_edges = {
        "follows": (inputs["follows_src"], inputs["follows_dst"], N_U, N_U),
        "rates": (inputs["rates_src"], inputs["rates_dst"], N_U, N_I),
        "rev": (inputs["rev_src"], inputs["rev_dst"], N_I, N_U),
    }
    scheds = {}
    for r, (s, d, ns, nd) in rel_edges.items():
        sched, packed = prep_relation(np.asarray(s), np.asarray(d), ns, nd)
        scheds[r] = (sched, packed)

    nc = build_program(cfg, scheds)

    iotaw = np.broadcast_to(np.arange(W, dtype=np.float16), (P, W)).copy()
    ident16 = np.eye(P, dtype=np.float16)
    ident32 = np.eye(P, dtype=np.float32)
    common = {}
    for n in ["x_user", "x_item",
              "W1_follows", "W1_rates", "W1_rev", "W2_follows", "W2_rates", "W2_rev"]:
        common[n] = np.asarray(inputs[n]).astype(np.float16)
    for n in ["b1_follows", "b1_rates", "b1_rev", "b2_follows", "b2_rates", "b2_rev"]:
        common[n] = np.asarray(inputs[n]).astype(np.float32)
    abl_idx0 = os.environ.get("ABL_IDX0") == "1"
    in_maps = []
    for k in range(NCORES):
        m = dict(common, iotaw=iotaw, ident16=ident16, ident32=ident32)
        for r in rel_edges:
            idx16, dstwA, normA = scheds[r][1][k]
            if abl_idx0:
                idx16 = np.zeros_like(idx16)
            m[f"idx_{r}"] = idx16
            m[f"dstw_{r}"] = dstwA
            m[f"norm_{r}"] = normA
        in_maps.append(m)
    return nc, in_maps


def assemble(results):
    u2 = np.concatenate([results[k]["out_user"] for k in range(NCORES)], axis=0)
    i2 = np.concatenate([results[k]["out_item"] for k in range(NCORES)], axis=0)
    return np.concatenate([u2, i2], axis=0)


def kernel(**inputs):
    nc, in_maps = prepare(inputs)
    res = run_bass_kernel_spmd(nc, in_maps, list(range(NCORES)))
    return assemble(res.results)


if __name__ == "__main__":
    pass


# revision 16
# speedup vs baseline: 1.2832x; 1.1629x over previous
"""Trainium2 Bass kernel for the 2-layer heterogeneous GCN encoder.

Strategy (8 NeuronCores, SPMD), v2:
  - Shard each relation's edges by dst-node owner: core k owns user rows
    [k*12500,(k+1)*12500) and item rows [k*6250,(k+1)*6250).
  - Aggregate-then-transform: segment_sum(x[src]*norm, dst) @ W computed as
    per-window PE matmuls  agg[fin, dstw] += gathered[e, fin].T @ S[e, dstw]
    with S[e, d] = (dstw_e == d) * norm_e built on DVE (fp16, 4x mode).
  - Gathers: one bulk SWDGE dma_gather per (dst-window x 32k src-range)
    (int16 local indices), fp16 tables -> ~0.3-2.4us of Pool time per
    thousands of rows instead of ~1us per 128 rows with indirect DMA.
  - Everything in the edge path is fp16 (tables, gathered rows, S, W);
    PSUM accumulation f32; final outputs f32.
  - Layer-1 outputs are written fp16, AllGathered (Shared DRAM), reused as
    layer-2 gather tables.

Self-contained: hardcodes problem shapes; host does only index-side prep.
"""

import os
import sys

sys.path.insert(0, "/opt/trn_rl_repo")

import numpy as np

import concourse.bass as bass
import concourse.bacc as bacc
import concourse.mybir as mybir
import concourse.tile as tile
from concourse.bass_utils import run_bass_kernel_spmd

P = 128
W = 256  # dst rows per aggregation window
RANGE = 32768  # max rows addressable by int16 gather indices
NCORES = 8
F16 = mybir.dt.float16
F32 = mybir.dt.float32
I16 = mybir.dt.int16

CFG = dict(N_U=100000, N_I=50000, E=1600000, D=128)

# relation -> (src table, dst type)
RELS = {
    "follows": ("user", "user"),
    "rates": ("user", "item"),
    "rev": ("item", "user"),
}


def _cdiv(a, b):
    return (a + b - 1) // b


class RelSched:
    """Harmonized per-(window, src-range) tile schedule for one relation."""

    def __init__(self, nwin, nrg):
        self.nwin = nwin
        self.nrg = nrg
        self.win = []  # per window: list of (rg, t0, T)
        self.Ttot = 0


def prep_relation(src, dst, n_src, n_dst, ncores=NCORES):
    """Shard edges by dst owner, sort by (dst window, src range), build the
    harmonized schedule and per-core packed arrays.

    Returns (sched, [(idx16, dstw, norm)] per core)."""
    shard = n_dst // ncores
    nwin = _cdiv(shard, W)
    nrg = _cdiv(n_src, RANGE)

    ones = np.ones_like(src, dtype=np.float64)
    deg_s = np.bincount(src, weights=ones, minlength=n_src)
    deg_d = np.bincount(dst, weights=ones, minlength=n_dst)
    inv_s = np.where(deg_s > 0, 1.0 / np.sqrt(deg_s), 0.0)
    inv_d = np.where(deg_d > 0, 1.0 / np.sqrt(deg_d), 0.0)
    norm = (inv_s[src] * inv_d[dst]).astype(np.float32)

    owner = dst // shard
    dloc = dst - owner * shard
    w_all = dloc // W
    dw_all = (dloc % W).astype(np.float32)
    rg_all = src // RANGE

    per_core = []
    counts = np.zeros((ncores, nwin * nrg), np.int64)
    for k in range(ncores):
        sel = owner == k
        s_k, dw_k, n_k = src[sel], dw_all[sel], norm[sel]
        cell = w_all[sel] * nrg + rg_all[sel]
        # sort by (cell, src): ascending gather addresses within each chunk
        order = np.lexsort((s_k, cell))
        s_k, dw_k, n_k, cell = s_k[order], dw_k[order], n_k[order], cell[order]
        counts[k] = np.bincount(cell, minlength=nwin * nrg)
        per_core.append((s_k, dw_k, n_k, cell))

    cmax = counts.max(axis=0).reshape(nwin, nrg)
    sched = RelSched(nwin, nrg)
    t0 = 0
    t0_cell = np.zeros((nwin, nrg), np.int64)
    for w in range(nwin):
        groups = []
        for rg in range(nrg):
            if cmax[w, rg] == 0:
                continue
            T = _cdiv(int(cmax[w, rg]), P)
            t0_cell[w, rg] = t0
            groups.append((rg, t0, T))
            t0 += T
        sched.win.append(groups)
    sched.Ttot = t0
    Ttot = t0

    packed = []
    for k in range(ncores):
        s_k, dw_k, n_k, cell = per_core[k]
        ck = counts[k]
        # position of each edge within its (w, rg) cell
        start = np.concatenate([[0], np.cumsum(ck)[:-1]])
        tok = np.arange(len(s_k)) - np.repeat(start, ck)
        # slot within the cell's tile block
        cw = cell // nrg
        crg = cell - cw * nrg
        base = t0_cell[cw, crg] * P
        slot = base + tok

        idx16 = np.zeros((16, Ttot * 8), np.int16)
        dstwA = np.full((P, Ttot), -1.0, np.float32)
        normA = np.zeros((P, Ttot), np.float32)
        loc = (s_k - crg * RANGE).astype(np.int16)
        idx16[slot % 16, slot // 16] = loc
        dstwA[slot % P, slot // P] = dw_k
        normA[slot % P, slot // P] = n_k
        packed.append((np.tile(idx16, (8, 1)), dstwA, normA))
    return sched, packed


def build_program(cfg, scheds):
    N_U, N_I, D = cfg["N_U"], cfg["N_I"], cfg["D"]
    SU, SI = N_U // NCORES, N_I // NCORES
    NWU, NWI = _cdiv(SU, W), _cdiv(SI, W)

    nq = int(os.environ.get("ABL_NQ", "1"))
    scratch = int(os.environ.get("ABL_SCRATCH", "16384"))
    nc = bacc.Bacc("TRN2", target_bir_lowering=False, num_swdge_queues=nq,
                   dynamic_dma_scratch_size=scratch)

    x_user = nc.dram_tensor("x_user", [N_U, D], F16, kind="ExternalInput")
    x_item = nc.dram_tensor("x_item", [N_I, D], F16, kind="ExternalInput")
    Ws = {
        n: nc.dram_tensor(n, [D, D], F16, kind="ExternalInput")
        for n in ["W1_follows", "W1_rates", "W1_rev", "W2_follows", "W2_rates", "W2_rev"]
    }
    bs = {
        n: nc.dram_tensor(n, [D], F32, kind="ExternalInput")
        for n in ["b1_follows", "b1_rates", "b1_rev", "b2_follows", "b2_rates", "b2_rev"]
    }
    iota_in = nc.dram_tensor("iotaw", [P, W], F16, kind="ExternalInput")
    ident16_in = nc.dram_tensor("ident16", [P, P], F16, kind="ExternalInput")
    ident32_in = nc.dram_tensor("ident32", [P, P], F32, kind="ExternalInput")
    streams = {}
    for r, (sched, _) in scheds.items():
        Ttot = sched.Ttot
        streams[r] = dict(
            idx=nc.dram_tensor(f"idx_{r}", [P, Ttot * 8], I16, kind="ExternalInput"),
            dstw=nc.dram_tensor(f"dstw_{r}", [P, Ttot], F32, kind="ExternalInput"),
            norm=nc.dram_tensor(f"norm_{r}", [P, Ttot], F32, kind="ExternalInput"),
        )
    out_user = nc.dram_tensor("out_user", [SU, D], F32, kind="ExternalOutput")
    out_item = nc.dram_tensor("out_item", [SI, D], F32, kind="ExternalOutput")

    with tile.TileContext(nc) as tc:
        with (
            tc.tile_pool(name="const", bufs=1) as cp,
            tc.tile_pool(name="gsl", bufs=3) as gp,
            tc.tile_pool(name="ixc", bufs=6) as ixp,
            tc.tile_pool(name="Sp", bufs=8) as sp,
            tc.tile_pool(name="agg", bufs=4) as aggp,
            tc.tile_pool(name="hb", bufs=3) as hp,
            tc.tile_pool(name="outp", bufs=4) as outp,
            tc.tile_pool(name="ps", bufs=3, space="PSUM") as pp,
            tc.tile_pool(name="psh", bufs=2, space="PSUM") as pph,
            tc.tile_pool(name="pstr", bufs=3, space="PSUM") as ptr,
            tc.tile_pool(name="dram", bufs=1, space="DRAM") as dp,
        ):
            # ---- constants ----
            iota_t = cp.tile([P, W], F16, tag="iota")
            nc.sync.dma_start(iota_t[:], iota_in[:])
            id16_t = cp.tile([P, P], F16, tag="id16")
            nc.sync.dma_start(id16_t[:], ident16_in[:])
            id32_t = cp.tile([P, P], F32, tag="id32")
            nc.sync.dma_start(id32_t[:], ident32_in[:])
            W_t = {}
            for n, Wd in Ws.items():
                W_t[n] = cp.tile([P, P], F16, tag=f"W_{n}", name=f"W_{n}")
                nc.sync.dma_start(W_t[n][:], Wd[:])
            b_t = {}
            for n, b in bs.items():
                b_t[n] = cp.tile([P, 1], F32, tag=f"b_{n}", name=f"bt_{n}")
                nc.sync.dma_start(b_t[n][:], b[:].unsqueeze(1))
            b1uv = cp.tile([P, 1], F32, tag="b1uv")
            nc.vector.tensor_tensor(
                out=b1uv[:], in0=b_t["b1_follows"][:], in1=b_t["b1_rev"][:],
                op=mybir.AluOpType.add,
            )
            nc.vector.tensor_scalar_mul(b1uv[:], b1uv[:], 0.5)
            b2uv = cp.tile([P, 1], F32, tag="b2uv")
            nc.vector.tensor_tensor(
                out=b2uv[:], in0=b_t["b2_follows"][:], in1=b_t["b2_rev"][:],
                op=mybir.AluOpType.add,
            )
            nc.vector.tensor_scalar_mul(b2uv[:], b2uv[:], 0.5)

            # ---- per-relation dstw/norm resident in SBUF ----
            st = {}
            for r, (sched, _) in scheds.items():
                Ttot = sched.Ttot
                st[r] = dict(
                    dstw=cp.tile([P, Ttot], F32, tag=f"dstw_{r}", name=f"dstwt_{r}"),
                    norm=cp.tile([P, Ttot], F32, tag=f"norm_{r}", name=f"normt_{r}"),
                )
                nc.sync.dma_start(st[r]["dstw"][:], streams[r]["dstw"][:])
                nc.sync.dma_start(st[r]["norm"][:], streams[r]["norm"][:])

            # ---- DRAM tiles for inter-layer tables (allocated per rep) ----

            ABL_NOS = os.environ.get("ABL_NOS") == "1"
            ABL_NOGATHER = os.environ.get("ABL_NOGATHER") == "1"
            ABL_NOMM = os.environ.get("ABL_NOMM") == "1"
            qctr = [0]

            def agg_window(rel, w, table_ap, n_table):
                """Aggregate window w of relation rel into a PSUM tile
                agg[fin=128, W] = sum_e x[src_e] (x) onehot(dstw_e)*norm_e."""
                sched = scheds[rel][0]
                groups = sched.win[w]
                Twin = sum(T for (_, _, T) in groups)
                assert Twin > 0
                gsl = gp.tile([P, Twin, P], F16, tag="gsl")
                j0 = 0
                tlist = []
                for rg, t0g, T in groups:
                    ic = ixp.tile([P, T * 8], I16, tag="ixc")
                    nc.sync.dma_start(
                        ic[:], streams[rel]["idx"][:, t0g * 8 : (t0g + T) * 8]
                    )
                    r0 = rg * RANGE
                    r1 = min(r0 + RANGE, n_table)
                    sp_tiles = int(os.environ.get("ABL_SP", "0"))
                    if not ABL_NOGATHER:
                        if sp_tiles > 0:
                            # chunk into <=sp_tiles-tile pieces, rr queues
                            spk = os.environ.get("ABL_SPKT", "1") == "1"
                            for c0 in range(0, T, sp_tiles):
                                c1 = min(c0 + sp_tiles, T)
                                qctr[0] = (qctr[0] + 1) % nq
                                nc.gpsimd.dma_gather(
                                    out_ap=gsl[:, j0 + c0 : j0 + c1, :],
                                    in_ap=table_ap[r0:r1, :],
                                    idxs_ap=ic[:, c0 * 8 : c1 * 8],
                                    num_idxs=(c1 - c0) * P,
                                    num_idxs_reg=(c1 - c0) * P,
                                    elem_size=P,
                                    single_packet=spk,
                                    queue_num=qctr[0],
                                )
                        else:
                            qctr[0] = (qctr[0] + 1) % nq
                            nc.gpsimd.dma_gather(
                                out_ap=gsl[:, j0 : j0 + T, :],
                                in_ap=table_ap[r0:r1, :],
                                idxs_ap=ic[:],
                                num_idxs=T * P,
                                num_idxs_reg=T * P,
                                elem_size=P,
                                single_packet=False,
                                queue_num=qctr[0],
                            )
                    for jj in range(T):
                        tlist.append((j0 + jj, t0g + jj))
                    j0 += T
                psum = pp.tile([P, W], F32, tag="aggps")
                for i, (j, t) in enumerate(tlist):
                    if ABL_NOS:
                        S = iota_t
                    else:
                        S = sp.tile([P, W], F16, tag="S")
                        nc.vector.tensor_scalar(
                            out=S[:],
                            in0=iota_t[:],
                            scalar1=st[rel]["dstw"][:, t : t + 1],
                            scalar2=st[rel]["norm"][:, t : t + 1],
                            op0=mybir.AluOpType.is_equal,
                            op1=mybir.AluOpType.mult,
                        )
                    if ABL_NOMM and not (i == 0 or i == len(tlist) - 1):
                        continue
                    nc.tensor.matmul(
                        out=psum[:],
                        lhsT=gsl[:, j, :],
                        rhs=S[:],
                        start=(i == 0),
                        stop=(i == len(tlist) - 1),
                    )
                return psum

            def write_windows(h_sb, w, nrows, dst_ap, ident_t, odt):
                """transpose h_sb [fout, nrows<=W] into [row, fout] blocks and
                DMA to dst_ap rows [w*W, w*W+nrows)."""
                for blk in range(_cdiv(nrows, P)):
                    r0, r1 = blk * P, min((blk + 1) * P, nrows)
                    ptile = ptr.tile([P, P], odt, tag="ptr")
                    nc.tensor.transpose(
                        out=ptile[: r1 - r0, :],
                        in_=h_sb[:, r0:r1],
                        identity=ident_t[:],
                    )
                    ob = outp.tile([P, P], odt, tag="ob")
                    nc.scalar.activation(
                        out=ob[: r1 - r0, :], in_=ptile[: r1 - r0, :],
                        func=mybir.ActivationFunctionType.Copy,
                    )
                    nc.sync.dma_start(
                        dst_ap[w * W + r0 : w * W + r1, :], ob[: r1 - r0, :]
                    )

            def user_layer(l, table_u, n_u, table_i, n_i, dst_ap, shard_rows, relu):
                Wf = W_t[f"W{l}_follows"]
                Wv = W_t[f"W{l}_rev"]
                bias = b1uv if l == 1 else b2uv
                nw = _cdiv(shard_rows, W)
                for w in range(nw):
                    nrows = min(W, shard_rows - w * W)
                    psF = agg_window("follows", w, table_u, n_u)
                    aggF = aggp.tile([P, W], F16, tag="aggF")
                    nc.scalar.activation(
                        out=aggF[:], in_=psF[:], func=mybir.ActivationFunctionType.Copy
                    )
                    psV = agg_window("rev", w, table_i, n_i)
                    aggV = aggp.tile([P, W], F16, tag="aggV")
                    nc.scalar.activation(
                        out=aggV[:], in_=psV[:], func=mybir.ActivationFunctionType.Copy
                    )
                    ph = pph.tile([P, W], F32, tag="hps")
                    nc.tensor.matmul(out=ph[:], lhsT=Wf[:], rhs=aggF[:], start=True, stop=False)
                    nc.tensor.matmul(out=ph[:], lhsT=Wv[:], rhs=aggV[:], start=False, stop=True)
                    if relu:
                        h_sb = hp.tile([P, W], F16, tag="h16")
                        nc.scalar.activation(
                            out=h_sb[:], in_=ph[:],
                            func=mybir.ActivationFunctionType.Relu,
                            bias=bias[:], scale=0.5,
                        )
                        write_windows(h_sb, w, nrows, dst_ap, id16_t, F16)
                    else:
                        h_sb = hp.tile([P, W], F32, tag="h32")
                        nc.vector.tensor_scalar(
                            out=h_sb[:], in0=ph[:],
                            scalar1=0.5, scalar2=bias[:],
                            op0=mybir.AluOpType.mult, op1=mybir.AluOpType.add,
                        )
                        write_windows(h_sb, w, nrows, dst_ap, id32_t, F32)

            def item_layer(l, table_u, n_u, dst_ap, shard_rows, relu):
                Wr = W_t[f"W{l}_rates"]
                bias = b_t[f"b{l}_rates"]
                nw = _cdiv(shard_rows, W)
                for w in range(nw):
                    nrows = min(W, shard_rows - w * W)
                    psR = agg_window("rates", w, table_u, n_u)
                    aggR = aggp.tile([P, W], F16, tag="aggR")
                    nc.scalar.activation(
                        out=aggR[:], in_=psR[:], func=mybir.ActivationFunctionType.Copy
                    )
                    ph = pph.tile([P, W], F32, tag="hps")
                    nc.tensor.matmul(out=ph[:], lhsT=Wr[:], rhs=aggR[:], start=True, stop=True)
                    if relu:
                        h_sb = hp.tile([P, W], F16, tag="h16")
                        nc.scalar.activation(
                            out=h_sb[:], in_=ph[:],
                            func=mybir.ActivationFunctionType.Relu,
                            bias=bias[:], scale=1.0,
                        )
                        write_windows(h_sb, w, nrows, dst_ap, id16_t, F16)
                    else:
                        h_sb = hp.tile([P, W], F32, tag="h32")
                        nc.vector.tensor_scalar(
                            out=h_sb[:], in0=ph[:],
                            scalar1=1.0, scalar2=bias[:],
                            op0=mybir.AluOpType.mult, op1=mybir.AluOpType.add,
                        )
                        write_windows(h_sb, w, nrows, dst_ap, id32_t, F32)

            ABL_NOAG = os.environ.get("ABL_NOAG") == "1"
            for _rep in range(int(os.environ.get("ABL_REPS", "1"))):
                u_slice = dp.tile([SU, D], F16, tag="u_slice", name=f"u_slice{_rep}")
                it_slice = dp.tile([SI, D], F16, tag="it_slice", name=f"it_slice{_rep}")
                u_full = dp.tile(
                    [N_U, D], F16, tag=f"u_full{_rep}", name=f"u_full{_rep}",
                    addr_space="Shared",
                )
                it_full = dp.tile(
                    [N_I, D], F16, tag=f"it_full{_rep}", name=f"it_full{_rep}",
                    addr_space="Shared",
                )
                # ---- layer 1 ----
                user_layer(1, x_user.ap(), N_U, x_item.ap(), N_I, u_slice[:], SU, relu=True)
                if not ABL_NOAG: nc.gpsimd.collective_compute(
                    "AllGather",
                    mybir.AluOpType.bypass,
                    replica_groups=[list(range(NCORES))],
                    ins=[u_slice[:]],
                    outs=[u_full[:]],
                )
                item_layer(1, x_user.ap(), N_U, it_slice[:], SI, relu=True)
                if not ABL_NOAG: nc.gpsimd.collective_compute(
                    "AllGather",
                    mybir.AluOpType.bypass,
                    replica_groups=[list(range(NCORES))],
                    ins=[it_slice[:]],
                    outs=[it_full[:]],
                )
                # ---- layer 2 (rates first: only needs u_full) ----
                item_layer(2, u_full[:], N_U, out_item.ap(), SI, relu=False)
                user_layer(2, u_full[:], N_U, it_full[:], N_I, out_user.ap(), SU, relu=False)

    nc.compile()
    return nc


def prepare(inputs):
    """Host-side prep + program build. Returns (nc, in_maps)."""
    cfg = dict(CFG)
    N_U = inputs["x_user"].shape[0]
    N_I = inputs["x_item"].shape[0]
    cfg.update(N_U=N_U, N_I=N_I, E=len(inputs["follows_src"]))

    rel_edges = {
        "follows": (inputs["follows_src"], inputs["follows_dst"], N_U, N_U),
        "rates": (inputs["rates_src"], inputs["rates_dst"], N_U, N_I),
        "rev": (inputs["rev_src"], inputs["rev_dst"], N_I, N_U),
    }
    scheds = {}
    for r, (s, d, ns, nd) in rel_edges.items():
        sched, packed = prep_relation(np.asarray(s), np.asarray(d), ns, nd)
        scheds[r] = (sched, packed)

    nc = build_program(cfg, scheds)

    iotaw = np.broadcast_to(np.arange(W, dtype=np.float16), (P, W)).copy()
    ident16 = np.eye(P, dtype=np.float16)
    ident32 = np.eye(P, dtype=np.float32)
    common = {}
    for n in ["x_user", "x_item",
              "W1_follows", "W1_rates", "W1_rev", "W2_follows", "W2_rates", "W2_rev"]:
        common[n] = np.asarray(inputs[n]).astype(np.float16)
    for n in ["b1_follows", "b1_rates", "b1_rev", "b2_follows", "b2_rates", "b2_rev"]:
        common[n] = np.asarray(inputs[n]).astype(np.float32)
    abl_idx0 = os.environ.get("ABL_IDX0") == "1"
    in_maps = []
    for k in range(NCORES):
        m = dict(common, iotaw=iotaw, ident16=ident16, ident32=ident32)
        for r in rel_edges:
            idx16, dstwA, normA = scheds[r][1][k]
            if abl_idx0:
                idx16 = np.zeros_like(idx16)
            m[f"idx_{r}"] = idx16
            m[f"dstw_{r}"] = dstwA
            m[f"norm_{r}"] = normA
        in_maps.append(m)
    return nc, in_maps


def assemble(results):
    u2 = np.concatenate([results[k]["out_user"] for k in range(NCORES)], axis=0)
    i2 = np.concatenate([results[k]["out_item"] for k in range(NCORES)], axis=0)
    return np.concatenate([u2, i2], axis=0)


def kernel(**inputs):
    nc, in_maps = prepare(inputs)
    res = run_bass_kernel_spmd(nc, in_maps, list(range(NCORES)))
    return assemble(res.results)


if __name__ == "__main__":
    pass
